# revision 11
# baseline (speedup 1.0000x reference)
"""ContextMatching kernel for Trainium2, 8-core SPMD — v3.

See kernel_v2 docstring for the math.  v3 changes vs v2 (44.8us):
  * p2 row-sums spread across three engines: K_X chunks as fused STT on DVE,
    K_Y as DVE tensor_tensor (2x) + ACT accum copy, K_Z as STT on GpSimd.
    The K_Y products are batched two chunks per DVE op (amortize op floor).
  * Softmax denominator: one DVE reduce (EBR) + one PE matmul at the end,
    replacing 10 per-chunk d-matmuls (PE was near-critical).
  * Store predicates: one multi-register reg_load per engine (v2 re-issued
    ~1us TENSOR_LOADs per cond between store issues).
  * cbs0 built on ACT, cbs1 on GpSimd (off the critical DVE path); b0 stores
    issue from scalar right after cbs0, b1 stores from sync.
  * Loads alternate between the sync and scalar HWDGE rings.
  * Cores sorted so core 0 gets the lightest store load.
  * partition-id preamble load disabled.
  * K_MM1024=1: single 1024-wide ctx matmul per chunk (PSUM 2 banks).
"""

import os

import numpy as np
import ml_dtypes

BF16 = ml_dtypes.bfloat16

B, T1, T2, C = 16, 1024, 1024, 1024
N_CORES = 8
BPC = B // N_CORES
P = 128
NRO = T1 // P

_cached = {}
last_results = None


def _build_program(NT):
    import concourse.mybir as mybir
    import concourse.tile as tile
    from concourse import bacc

    f32 = mybir.dt.float32
    bf16 = mybir.dt.bfloat16
    fp8 = mybir.dt.float8e4
    i32 = mybir.dt.int32
    Alu = mybir.AluOpType
    Act = mybir.ActivationFunctionType
    Axis = mybir.AxisListType

    SG = int(os.environ.get("K_SG", "2"))
    NG = NRO // SG
    KX = int(os.environ.get("K_X", "4"))  # chunks via DVE STT (fused)
    KZ = int(os.environ.get("K_Z", "0"))  # chunks: GpSimd product + ACT accum
    MM1024 = os.environ.get("K_MM1024", "0") == "1"
    MG = os.environ.get("K_MG", "0") == "1"
    TTR = os.environ.get("K_TTR", "0") == "1"  # chunk-0 reduction via tensor_tensor_reduce
    NOPID = os.environ.get("K_PID", "1") == "0"  # partition_id required under axon/bass2jax

    # y chunks (DVE product + ACT accum) first so ACT starts early and stays
    # fed; x chunks (fused DVE STT) last.  z unused (gpsimd too slow).
    KZ = min(KZ, NT)
    KX = min(KX, NT - KZ)
    z_set = set()
    x_set = set(range(NT - KX, NT))

    MC = C + 2 * NT + 1 + 2 * P
    O_AUX = C
    O_ONE = C + 2 * NT
    O_SEL = C + 2 * NT + 1

    nc = bacc.Bacc(
        None,
        target_bir_lowering=False,
        name="context_matching3",
        enable_partition_id=not NOPID,
    )

    s2cd = nc.dram_tensor("s2c", [P, NT * C], bf16, kind="ExternalInput")
    metad = nc.dram_tensor("meta", [P, MC], bf16, kind="ExternalInput")
    cndd = nc.dram_tensor("cnd", [1, 2 * NG], i32, kind="ExternalInput")
    outd = [
        nc.dram_tensor(f"out{b}", [T1, C], bf16, kind="ExternalOutput")
        for b in range(BPC)
    ]

    pairs = [list(range(g, min(g + 2, NT))) for g in range(0, NT, 2)]
    # mask groups: small first so PE starts early, bigger later
    mgroups = []
    t = 0
    for sz in (1, 1, 2, 2, 3, 4, 4):
        if t >= NT:
            break
        mgroups.append(list(range(t, min(t + sz, NT))))
        t += sz

    with tile.TileContext(nc) as tc:
        with (
            tc.tile_pool(name="statics", bufs=1) as statics,
            tc.tile_pool(name="s2pool", bufs=len(pairs)) as s2pool,
            tc.tile_pool(name="scra", bufs=3) as scra,
            tc.tile_pool(name="scrb", bufs=2) as scrb,
            tc.tile_pool(name="smalls", bufs=1) as smalls,
            tc.tile_pool(name="pctx", bufs=1, space="PSUM") as pctx,
            tc.tile_pool(name="pcb", bufs=1, space="PSUM") as pcb,
        ):
            # ---- loads: statics first; s2 groups alternate the two rings ----
            meta = statics.tile([P, MC], bf16)
            nc.sync.dma_start(out=meta, in_=metad[:, :])
            cndt = statics.tile([1, 2 * NG], i32)

            w2b = meta[:, 0:C]
            ones_c = meta[:, O_ONE : O_ONE + 1]
            sel = [meta[0:2, O_SEL + b * P : O_SEL + (b + 1) * P] for b in range(BPC)]

            s2ts = []
            for gi, grp in enumerate(pairs):
                gl = len(grp)
                s2t = s2pool.tile([P, gl * C], bf16, name=f"s2t_{gi}", tag="s2t")
                eng = nc.sync if gi % 2 == 0 else nc.scalar
                eng.dma_start(out=s2t, in_=s2cd[:, grp[0] * C : (grp[-1] + 1) * C])
                s2ts.append(s2t)
            nc.scalar.dma_start(out=cndt, in_=cndd[:, :])

            # ---- store predicates: one multi-reg load per engine ----
            cregs = []
            for b in range(BPC):
                eng = nc.scalar if b == 0 else nc.sync
                regs = [eng.alloc_register(f"cnd_{b}_{w}") for w in range(NG)]
                eng.reg_load(regs, cndt[0:1, b * NG : (b + 1) * NG])
                cregs.append([eng.snap(r, min_val=0, max_val=1) for r in regs])

            # ---- persistent smalls ----
            p2f = smalls.tile([P, NT], f32, name="p2f")
            e = smalls.tile([P, NT], bf16, name="e")
            E2 = smalls.tile([P, NT, 2], bf16, name="E2")
            EBR = smalls.tile([P, 2], f32, name="EBR")
            rinv2 = smalls.tile([2, 1], f32, name="rinv2")
            ctxs = smalls.tile([2, C], bf16, name="ctxs")
            cbs = [smalls.tile([P, C], bf16, name=f"cbs_{b}") for b in range(BPC)]

            ctxp = pctx.tile([2, C], f32, name="ctxp")
            d2p = pctx.tile([2, 1], f32, name="d2p")
            ones_f = smalls.tile([P, 1], f32, name="ones_f")
            nc.gpsimd.memset(ones_f, 1.0)

            # ---- p2 products/row-sums, engine-split ----
            done_e = set()
            mg_i = 0
            started = False
            for gi, grp in enumerate(pairs):
                s2t = s2ts[gi]
                ys = [t for t in grp if t not in z_set and t not in x_set]
                # batched product for the ACT-accum chunks of this pair
                if len(ys) == 2:
                    sa = scra.tile([P, 2 * C], bf16, name=f"sa_{grp[0]}", tag="sa")
                    nc.vector.tensor_tensor(
                        out=sa.rearrange("p (t c) -> p t c", t=2),
                        in0=s2t[:, 0 : 2 * C].rearrange("p (t c) -> p t c", t=2),
                        in1=w2b.unsqueeze(1).broadcast_to([P, 2, C]),
                        op=Alu.mult,
                    )
                    for k, t in enumerate(grp):
                        sb = scrb.tile([P, C], fp8, name=f"sb_{t}", tag="sb")
                        nc.scalar.activation(
                            out=sb, in_=sa[:, k * C : (k + 1) * C],
                            func=Act.Copy, accum_out=p2f[:, t : t + 1],
                        )
                else:
                    for k, t in enumerate(grp):
                        sl = s2t[:, k * C : (k + 1) * C]
                        if t in z_set:
                            sa = scra.tile([P, C], bf16, name=f"sa_{t}", tag="sa")
                            nc.gpsimd.tensor_mul(sa, sl, w2b)
                            sb = scrb.tile([P, C], fp8, name=f"sb_{t}", tag="sb")
                            nc.scalar.activation(
                                out=sb, in_=sa, func=Act.Copy,
                                accum_out=p2f[:, t : t + 1],
                            )
                        elif t in x_set:
                            sa = scra.tile([P, C], fp8, name=f"sa_{t}", tag="sa")
                            if TTR and t == 0:
                                nc.vector.tensor_tensor_reduce(
                                    out=sa, in0=sl, in1=w2b, scale=1.0,
                                    scalar=0.0, op0=Alu.mult, op1=Alu.add,
                                    accum_out=p2f[:, t : t + 1],
                                )
                            else:
                                nc.vector.scalar_tensor_tensor(
                                    out=sa, in0=sl, scalar=1.0, in1=w2b,
                                    op0=Alu.mult, op1=Alu.mult,
                                    accum_out=p2f[:, t : t + 1],
                                )
                        else:
                            sa = scra.tile([P, C], bf16, name=f"sa_{t}", tag="sa")
                            nc.vector.tensor_tensor(out=sa, in0=sl, in1=w2b,
                                                    op=Alu.mult)
                            sb = scrb.tile([P, C], fp8, name=f"sb_{t}", tag="sb")
                            nc.scalar.activation(
                                out=sb, in_=sa, func=Act.Copy,
                                accum_out=p2f[:, t : t + 1],
                            )

                # exp + mask + matmuls for any completed mask-group
                done_e.update(grp)
                while mg_i < len(mgroups) and all(t in done_e for t in mgroups[mg_i]):
                    mg = mgroups[mg_i]
                    g0, g1 = mg[0], mg[-1] + 1
                    nc.scalar.activation(
                        out=e[:, g0:g1], in_=p2f[:, g0:g1], func=Act.Exp
                    )
                    meng = nc.gpsimd if MG else nc.vector
                    meng.tensor_mul(
                        E2[:, g0:g1, :],
                        e[:, g0:g1].unsqueeze(2).broadcast_to([P, g1 - g0, 2]),
                        meta[:, O_AUX + 2 * g0 : O_AUX + 2 * g1].rearrange(
                            "p (t two) -> p t two", two=2
                        ),
                    )
                    for t in mg:
                        gj, kj_ = t // 2, t % 2
                        st = not started
                        started = True
                        sp = t == NT - 1
                        if MM1024:
                            nc.tensor.matmul(
                                ctxp[:, :], lhsT=E2[:, t, :],
                                rhs=s2ts[gj][:, kj_ * C : (kj_ + 1) * C],
                                start=st, stop=sp,
                            )
                        else:
                            for h in range(2):
                                nc.tensor.matmul(
                                    ctxp[:, h * 512 : (h + 1) * 512],
                                    lhsT=E2[:, t, :],
                                    rhs=s2ts[gj][
                                        :, kj_ * C + h * 512 : kj_ * C + (h + 1) * 512
                                    ],
                                    start=st, stop=sp,
                                )
                    mg_i += 1

            # ---- d = sum(E2) per batch: DVE reduce + one PE matmul ----
            nc.vector.tensor_reduce(
                out=EBR,
                in_=E2.rearrange("p t b -> p b t"),
                axis=Axis.X,
                op=Alu.add,
            )
            nc.tensor.matmul(d2p, lhsT=EBR, rhs=ones_f, start=True, stop=True)
            nc.vector.reciprocal(rinv2, d2p)

            # ---- ctxs = rinv * ctx (f32->bf16), halves on ACT / DVE ----
            nc.scalar.activation(
                out=ctxs[:, 0:512], in_=ctxp[:, 0:512], func=Act.Copy,
                scale=rinv2[:, 0:1],
            )
            nc.vector.tensor_scalar_mul(
                out=ctxs[:, 512:1024], in0=ctxp[:, 512:1024], scalar1=rinv2[:, 0:1]
            )

            # ---- broadcast to 128 partitions; cbs0 via ACT, cbs1 via GpSimd ----
            for b in range(BPC):
                cbp = pcb.tile([P, C], f32, name=f"cbp_{b}", tag=f"cbp{b}")
                if MM1024:
                    nc.tensor.matmul(cbp[:, :], lhsT=sel[b], rhs=ctxs[:, :],
                                     start=True, stop=True)
                else:
                    for h in range(2):
                        cols = slice(h * 512, (h + 1) * 512)
                        nc.tensor.matmul(cbp[:, cols], lhsT=sel[b],
                                         rhs=ctxs[:, cols], start=True, stop=True)
                if b == 0:
                    nc.scalar.activation(out=cbs[b], in_=cbp, func=Act.Copy)
                else:
                    nc.vector.tensor_copy(out=cbs[b], in_=cbp)

            # ---- predicated contiguous stores (b0 on scalar, b1 on sync) ----
            for b in range(BPC):
                eng = nc.scalar if b == 0 else nc.sync
                src = cbs[b].unsqueeze(1).broadcast_to([P, SG, C])
                ov = outd[b].rearrange("(w g p) c -> w p g c", g=SG, p=P)
                for w in range(NG):
                    eng.dma_start(out=ov[w], in_=src, cond=cregs[b][w])

    nc.finalize()
    return nc


def _plan(l1, l2):
    """Pair batches minimizing (NT, max store chunks); core order by store load."""
    kj = (-(-l2 // P)).astype(np.int64)
    ki = (-(-l1 // P)).astype(np.int64)
    n = len(kj)
    from functools import lru_cache

    @lru_cache(maxsize=None)
    def best(mask):
        if mask == 0:
            return (0, 0, ())
        lo = (mask & -mask).bit_length() - 1
        rest = mask ^ (1 << lo)
        res = None
        mm = rest
        while mm:
            j = (mm & -mm).bit_length() - 1
            mm ^= 1 << j
            sub = best(rest ^ (1 << j))
            cand = (
                max(int(kj[lo] + kj[j]), sub[0]),
                max(int(ki[lo] + ki[j]), sub[1]),
                ((lo, j),) + sub[2],
            )
            if res is None or cand[:2] < res[:2]:
                res = cand
        return res

    nt, _, prs = best((1 << n) - 1)
    pairs = [(a, b) if l1[a] >= l1[b] else (b, a) for a, b in prs]
    pairs.sort(key=lambda p: int(ki[p[0]] + ki[p[1]]))  # core 0 lightest stores
    return pairs, kj, int(nt)


def kernel(s1, l1, s2, l2, w):
    global last_results
    from concourse.bass_utils import run_bass_kernel_spmd

    s2 = np.asarray(s2)
    w = np.asarray(w, dtype=np.float32)
    l1 = np.asarray(l1).astype(np.int64).ravel()
    l2 = np.asarray(l2).astype(np.int64).ravel()
    assert s2.shape == (B, T2, C) and w.shape == (1, 2 * C)

    SG = int(os.environ.get("K_SG", "2"))
    NG = NRO // SG

    pairs, kj, NT = _plan(l1, l2)
    if NT not in _cached:
        _cached[NT] = _build_program(NT)
    nc = _cached[NT]

    MC = C + 2 * NT + 1 + 2 * P
    O_AUX = C
    O_ONE = C + 2 * NT
    O_SEL = C + 2 * NT + 1

    meta = np.zeros((P, MC), dtype=BF16)
    meta[:, 0:C] = np.broadcast_to(w[0, C:].astype(BF16), (P, C))
    meta[:, O_ONE] = 1.0
    for b in range(BPC):
        meta[b, O_SEL + b * P : O_SEL + (b + 1) * P] = 1.0
    iot = np.arange(P)

    in_maps = []
    for c in range(N_CORES):
        s2c = np.zeros((P, NT * C), dtype=BF16)
        m = meta.copy()
        cnd = np.zeros((1, 2 * NG), dtype=np.int32)
        base_t = 0
        for lb, g in enumerate(pairs[c]):
            for k in range(int(kj[g])):
                t = base_t + k
                j0 = k * P
                s2c[:, t * C : (t + 1) * C] = s2[g, j0 : j0 + P, :]
                m[:, O_AUX + 2 * t + lb] = (j0 + iot) < l2[g]
            base_t += int(kj[g])
            cnd[0, lb * NG : (lb + 1) * NG] = (
                np.arange(NG) * SG * P < l1[g]
            ).astype(np.int32)
        in_maps.append({"s2c": s2c, "meta": m, "cnd": cnd})

    last_results = run_bass_kernel_spmd(nc, in_maps, core_ids=list(range(N_CORES)))

    out = np.zeros((B, T1, C), dtype=np.float32)
    for c in range(N_CORES):
        for lb, g in enumerate(pairs[c]):
            nv = int(l1[g])
            res = last_results.results[c][f"out{lb}"]
            out[g, :nv] = res[:nv].astype(np.float32)
    return out


# revision 12
# speedup vs baseline: 1.0086x; 1.0086x over previous
"""ContextMatching kernel for Trainium2, 8-core SPMD — v3.

See kernel_v2 docstring for the math.  v3 changes vs v2 (44.8us):
  * p2 row-sums spread across three engines: K_X chunks as fused STT on DVE,
    K_Y as DVE tensor_tensor (2x) + ACT accum copy, K_Z as STT on GpSimd.
    The K_Y products are batched two chunks per DVE op (amortize op floor).
  * Softmax denominator: one DVE reduce (EBR) + one PE matmul at the end,
    replacing 10 per-chunk d-matmuls (PE was near-critical).
  * Store predicates: one multi-register reg_load per engine (v2 re-issued
    ~1us TENSOR_LOADs per cond between store issues).
  * cbs0 built on ACT, cbs1 on GpSimd (off the critical DVE path); b0 stores
    issue from scalar right after cbs0, b1 stores from sync.
  * Loads alternate between the sync and scalar HWDGE rings.
  * Cores sorted so core 0 gets the lightest store load.
  * partition-id preamble load disabled.
  * K_MM1024=1: single 1024-wide ctx matmul per chunk (PSUM 2 banks).
"""

import os

import numpy as np
import ml_dtypes

BF16 = ml_dtypes.bfloat16

B, T1, T2, C = 16, 1024, 1024, 1024
N_CORES = 8
BPC = B // N_CORES
P = 128
NRO = T1 // P

_cached = {}
last_results = None


def _build_program(NT):
    import concourse.mybir as mybir
    import concourse.tile as tile
    from concourse import bacc

    f32 = mybir.dt.float32
    bf16 = mybir.dt.bfloat16
    fp8 = mybir.dt.float8e4
    i32 = mybir.dt.int32
    Alu = mybir.AluOpType
    Act = mybir.ActivationFunctionType
    Axis = mybir.AxisListType

    SG = int(os.environ.get("K_SG", "2"))
    NG = NRO // SG
    KX = int(os.environ.get("K_X", "4"))  # chunks via DVE STT (fused)
    KZ = int(os.environ.get("K_Z", "0"))  # chunks: GpSimd product + ACT accum
    MM1024 = os.environ.get("K_MM1024", "0") == "1"
    MG = os.environ.get("K_MG", "0") == "1"
    TTR = os.environ.get("K_TTR", "0") == "1"  # chunk-0 reduction via tensor_tensor_reduce
    NOPID = os.environ.get("K_PID", "1") == "0"  # partition_id required under axon/bass2jax

    # y chunks (DVE product + ACT accum) first so ACT starts early and stays
    # fed; x chunks (fused DVE STT) last.  z unused (gpsimd too slow).
    KZ = min(KZ, NT)
    KX = min(KX, NT - KZ)
    z_set = set()
    x_set = set(range(NT - KX, NT))

    MC = C + 2 * NT + 1 + 2 * P
    O_AUX = C
    O_ONE = C + 2 * NT
    O_SEL = C + 2 * NT + 1

    nc = bacc.Bacc(
        None,
        target_bir_lowering=False,
        name="context_matching3",
        enable_partition_id=not NOPID,
    )

    s2cd = nc.dram_tensor("s2c", [P, NT * C], bf16, kind="ExternalInput")
    metad = nc.dram_tensor("meta", [P, MC], bf16, kind="ExternalInput")
    cndd = nc.dram_tensor("cnd", [1, 2 * NG], i32, kind="ExternalInput")
    outd = [
        nc.dram_tensor(f"out{b}", [T1, C], bf16, kind="ExternalOutput")
        for b in range(BPC)
    ]

    pairs = [list(range(g, min(g + 2, NT))) for g in range(0, NT, 2)]
    # mask groups: small first so PE starts early, bigger later
    mgroups = []
    t = 0
    for sz in (1, 1, 2, 2, 3, 4, 4):
        if t >= NT:
            break
        mgroups.append(list(range(t, min(t + sz, NT))))
        t += sz

    with tile.TileContext(nc) as tc:
        with (
            tc.tile_pool(name="statics", bufs=1) as statics,
            tc.tile_pool(name="s2pool", bufs=len(pairs)) as s2pool,
            tc.tile_pool(name="scra", bufs=3) as scra,
            tc.tile_pool(name="scrb", bufs=2) as scrb,
            tc.tile_pool(name="smalls", bufs=1) as smalls,
            tc.tile_pool(name="pctx", bufs=1, space="PSUM") as pctx,
            tc.tile_pool(name="pcb", bufs=1, space="PSUM") as pcb,
        ):
            # ---- loads: statics first; s2 groups alternate the two rings ----
            meta = statics.tile([P, MC], bf16)
            nc.sync.dma_start(out=meta, in_=metad[:, :])
            cndt = statics.tile([1, 2 * NG], i32)

            w2b = meta[:, 0:C]
            ones_c = meta[:, O_ONE : O_ONE + 1]
            sel = [meta[0:2, O_SEL + b * P : O_SEL + (b + 1) * P] for b in range(BPC)]

            s2ts = []
            for gi, grp in enumerate(pairs):
                gl = len(grp)
                s2t = s2pool.tile([P, gl * C], bf16, name=f"s2t_{gi}", tag="s2t")
                eng = nc.sync if gi % 2 == 0 else nc.scalar
                eng.dma_start(out=s2t, in_=s2cd[:, grp[0] * C : (grp[-1] + 1) * C])
                s2ts.append(s2t)
            nc.scalar.dma_start(out=cndt, in_=cndd[:, :])

            # ---- store predicates: one multi-reg load per engine ----
            cregs = []
            for b in range(BPC):
                eng = nc.scalar if b == 0 else nc.sync
                regs = [eng.alloc_register(f"cnd_{b}_{w}") for w in range(NG)]
                eng.reg_load(regs, cndt[0:1, b * NG : (b + 1) * NG])
                cregs.append([eng.snap(r, min_val=0, max_val=1) for r in regs])

            # ---- persistent smalls ----
            p2f = smalls.tile([P, NT], f32, name="p2f")
            e = smalls.tile([P, NT], bf16, name="e")
            E2 = smalls.tile([P, NT, 2], bf16, name="E2")
            EBR = smalls.tile([P, 2], f32, name="EBR")
            rinv2 = smalls.tile([2, 1], f32, name="rinv2")
            ctxs = smalls.tile([2, C], bf16, name="ctxs")
            cbs = [smalls.tile([P, C], bf16, name=f"cbs_{b}") for b in range(BPC)]

            ctxp = pctx.tile([2, C], f32, name="ctxp")
            d2p = pctx.tile([2, 1], f32, name="d2p")
            ones_f = smalls.tile([P, 1], f32, name="ones_f")
            nc.gpsimd.memset(ones_f, 1.0)

            # ---- p2 products/row-sums, engine-split ----
            done_e = set()
            mg_i = 0
            started = False
            for gi, grp in enumerate(pairs):
                s2t = s2ts[gi]
                ys = [t for t in grp if t not in z_set and t not in x_set]
                # batched product for the ACT-accum chunks of this pair
                if len(ys) == 2:
                    sa = scra.tile([P, 2 * C], bf16, name=f"sa_{grp[0]}", tag="sa")
                    nc.vector.tensor_tensor(
                        out=sa.rearrange("p (t c) -> p t c", t=2),
                        in0=s2t[:, 0 : 2 * C].rearrange("p (t c) -> p t c", t=2),
                        in1=w2b.unsqueeze(1).broadcast_to([P, 2, C]),
                        op=Alu.mult,
                    )
                    for k, t in enumerate(grp):
                        sb = scrb.tile([P, C], fp8, name=f"sb_{t}", tag="sb")
                        nc.scalar.activation(
                            out=sb, in_=sa[:, k * C : (k + 1) * C],
                            func=Act.Copy, accum_out=p2f[:, t : t + 1],
                        )
                else:
                    for k, t in enumerate(grp):
                        sl = s2t[:, k * C : (k + 1) * C]
                        if t in z_set:
                            sa = scra.tile([P, C], bf16, name=f"sa_{t}", tag="sa")
                            nc.gpsimd.tensor_mul(sa, sl, w2b)
                            sb = scrb.tile([P, C], fp8, name=f"sb_{t}", tag="sb")
                            nc.scalar.activation(
                                out=sb, in_=sa, func=Act.Copy,
                                accum_out=p2f[:, t : t + 1],
                            )
                        elif t in x_set:
                            sa = scra.tile([P, C], fp8, name=f"sa_{t}", tag="sa")
                            if TTR and t == 0:
                                nc.vector.tensor_tensor_reduce(
                                    out=sa, in0=sl, in1=w2b, scale=1.0,
                                    scalar=0.0, op0=Alu.mult, op1=Alu.add,
                                    accum_out=p2f[:, t : t + 1],
                                )
                            else:
                                nc.vector.scalar_tensor_tensor(
                                    out=sa, in0=sl, scalar=1.0, in1=w2b,
                                    op0=Alu.mult, op1=Alu.mult,
                                    accum_out=p2f[:, t : t + 1],
                                )
                        else:
                            sa = scra.tile([P, C], bf16, name=f"sa_{t}", tag="sa")
                            nc.vector.tensor_tensor(out=sa, in0=sl, in1=w2b,
                                                    op=Alu.mult)
                            sb = scrb.tile([P, C], fp8, name=f"sb_{t}", tag="sb")
                            nc.scalar.activation(
                                out=sb, in_=sa, func=Act.Copy,
                                accum_out=p2f[:, t : t + 1],
                            )

                # exp + mask + matmuls for any completed mask-group
                done_e.update(grp)
                while mg_i < len(mgroups) and all(t in done_e for t in mgroups[mg_i]):
                    mg = mgroups[mg_i]
                    g0, g1 = mg[0], mg[-1] + 1
                    nc.scalar.activation(
                        out=e[:, g0:g1], in_=p2f[:, g0:g1], func=Act.Exp
                    )
                    meng = nc.gpsimd if MG else nc.vector
                    meng.tensor_mul(
                        E2[:, g0:g1, :],
                        e[:, g0:g1].unsqueeze(2).broadcast_to([P, g1 - g0, 2]),
                        meta[:, O_AUX + 2 * g0 : O_AUX + 2 * g1].rearrange(
                            "p (t two) -> p t two", two=2
                        ),
                    )
                    if mg[-1] == NT - 1:
                        nc.vector.tensor_reduce(
                            out=EBR,
                            in_=E2.rearrange("p t b -> p b t"),
                            axis=Axis.X,
                            op=Alu.add,
                        )
                        nc.tensor.matmul(d2p, lhsT=EBR, rhs=ones_f,
                                         start=True, stop=True)
                        nc.vector.reciprocal(rinv2, d2p)
                    for t in mg:
                        gj, kj_ = t // 2, t % 2
                        st = not started
                        started = True
                        sp = t == NT - 1
                        if MM1024:
                            nc.tensor.matmul(
                                ctxp[:, :], lhsT=E2[:, t, :],
                                rhs=s2ts[gj][:, kj_ * C : (kj_ + 1) * C],
                                start=st, stop=sp,
                            )
                        else:
                            for h in range(2):
                                nc.tensor.matmul(
                                    ctxp[:, h * 512 : (h + 1) * 512],
                                    lhsT=E2[:, t, :],
                                    rhs=s2ts[gj][
                                        :, kj_ * C + h * 512 : kj_ * C + (h + 1) * 512
                                    ],
                                    start=st, stop=sp,
                                )
                    mg_i += 1

            # ---- ctxs = rinv * ctx (f32->bf16), halves on ACT / DVE ----
            nc.scalar.activation(
                out=ctxs[:, 0:512], in_=ctxp[:, 0:512], func=Act.Copy,
                scale=rinv2[:, 0:1],
            )
            nc.vector.tensor_scalar_mul(
                out=ctxs[:, 512:1024], in0=ctxp[:, 512:1024], scalar1=rinv2[:, 0:1]
            )

            # ---- broadcast to 128 partitions; cbs0 via ACT, cbs1 via GpSimd ----
            for b in range(BPC):
                cbp = pcb.tile([P, C], f32, name=f"cbp_{b}", tag=f"cbp{b}")
                if MM1024:
                    nc.tensor.matmul(cbp[:, :], lhsT=sel[b], rhs=ctxs[:, :],
                                     start=True, stop=True)
                else:
                    for h in range(2):
                        cols = slice(h * 512, (h + 1) * 512)
                        nc.tensor.matmul(cbp[:, cols], lhsT=sel[b],
                                         rhs=ctxs[:, cols], start=True, stop=True)
                if b == 0:
                    nc.scalar.activation(out=cbs[b][:, 0:512], in_=cbp[:, 0:512],
                                         func=Act.Copy)
                    nc.vector.tensor_copy(out=cbs[b][:, 512:1024],
                                          in_=cbp[:, 512:1024])
                else:
                    nc.vector.tensor_copy(out=cbs[b][:, 0:512],
                                          in_=cbp[:, 0:512])
                    nc.scalar.activation(out=cbs[b][:, 512:1024],
                                         in_=cbp[:, 512:1024], func=Act.Copy)

            # ---- predicated contiguous stores (b0 on scalar, b1 on sync) ----
            for b in range(BPC):
                eng = nc.scalar if b == 0 else nc.sync
                src = cbs[b].unsqueeze(1).broadcast_to([P, SG, C])
                ov = outd[b].rearrange("(w g p) c -> w p g c", g=SG, p=P)
                for w in range(NG):
                    eng.dma_start(out=ov[w], in_=src, cond=cregs[b][w])

    nc.finalize()
    return nc


def _plan(l1, l2):
    """Pair batches minimizing (NT, max store chunks); core order by store load."""
    kj = (-(-l2 // P)).astype(np.int64)
    ki = (-(-l1 // P)).astype(np.int64)
    n = len(kj)
    from functools import lru_cache

    @lru_cache(maxsize=None)
    def best(mask):
        if mask == 0:
            return (0, 0, ())
        lo = (mask & -mask).bit_length() - 1
        rest = mask ^ (1 << lo)
        res = None
        mm = rest
        while mm:
            j = (mm & -mm).bit_length() - 1
            mm ^= 1 << j
            sub = best(rest ^ (1 << j))
            cand = (
                max(int(kj[lo] + kj[j]), sub[0]),
                max(int(ki[lo] + ki[j]), sub[1]),
                ((lo, j),) + sub[2],
            )
            if res is None or cand[:2] < res[:2]:
                res = cand
        return res

    nt, _, prs = best((1 << n) - 1)
    pairs = [(a, b) if l1[a] >= l1[b] else (b, a) for a, b in prs]
    pairs.sort(key=lambda p: int(ki[p[0]] + ki[p[1]]))  # core 0 lightest stores
    return pairs, kj, int(nt)


def kernel(s1, l1, s2, l2, w):
    global last_results
    from concourse.bass_utils import run_bass_kernel_spmd

    s2 = np.asarray(s2)
    w = np.asarray(w, dtype=np.float32)
    l1 = np.asarray(l1).astype(np.int64).ravel()
    l2 = np.asarray(l2).astype(np.int64).ravel()
    assert s2.shape == (B, T2, C) and w.shape == (1, 2 * C)

    SG = int(os.environ.get("K_SG", "2"))
    NG = NRO // SG

    pairs, kj, NT = _plan(l1, l2)
    if NT not in _cached:
        _cached[NT] = _build_program(NT)
    nc = _cached[NT]

    MC = C + 2 * NT + 1 + 2 * P
    O_AUX = C
    O_ONE = C + 2 * NT
    O_SEL = C + 2 * NT + 1

    meta = np.zeros((P, MC), dtype=BF16)
    meta[:, 0:C] = np.broadcast_to(w[0, C:].astype(BF16), (P, C))
    meta[:, O_ONE] = 1.0
    for b in range(BPC):
        meta[b, O_SEL + b * P : O_SEL + (b + 1) * P] = 1.0
    iot = np.arange(P)

    in_maps = []
    for c in range(N_CORES):
        s2c = np.zeros((P, NT * C), dtype=BF16)
        m = meta.copy()
        cnd = np.zeros((1, 2 * NG), dtype=np.int32)
        base_t = 0
        for lb, g in enumerate(pairs[c]):
            for k in range(int(kj[g])):
                t = base_t + k
                j0 = k * P
                s2c[:, t * C : (t + 1) * C] = s2[g, j0 : j0 + P, :]
                m[:, O_AUX + 2 * t + lb] = (j0 + iot) < l2[g]
            base_t += int(kj[g])
            cnd[0, lb * NG : (lb + 1) * NG] = (
                np.arange(NG) * SG * P < l1[g]
            ).astype(np.int32)
        in_maps.append({"s2c": s2c, "meta": m, "cnd": cnd})

    last_results = run_bass_kernel_spmd(nc, in_maps, core_ids=list(range(N_CORES)))

    out = np.zeros((B, T1, C), dtype=np.float32)
    for c in range(N_CORES):
        for lb, g in enumerate(pairs[c]):
            nv = int(l1[g])
            res = last_results.results[c][f"out{lb}"]
            out[g, :nv] = res[:nv].astype(np.float32)
    return out
